# revision 38
# baseline (speedup 1.0000x reference)
"""Trainium2 Bass kernel for nn_Attention_56487409877769.

NdLinear-qkv -> 16-head attention -> NdLinear-proj, B=4 N=1024 C=1024 H=16.

Sharding: 8 cores = (batch b, head-group g) with b=core//2, g=core%2.
Each core handles batch b and its 8 heads (qkv channel slice 512g:512g+512).
The proj channel matmul is a partial sum over the core's channel slice; the
host adds the two partials per batch plus a rank-1 bias term (the NdLinear
proj biases commute: out = Wp0 @ O @ Wp1c.T + outer(bp0, Wp1.sum(1)) + bp1).

v3 design notes:
- On-device dtype-SIZE-converting writes (fp32 PSUM -> fp16/fp8 SBUF) are
  broken for downstream PE consumers in this stack (verified by minimal
  repro; DMA readers see such tiles fine).  Therefore every PE-consumed
  tile that is produced on-device is fp32r (same-size as its fp32 source);
  narrow dtypes are used only where the data is HOST-prepared and
  DMA-written (phase-A fp8 DoubleRow inputs) or DMA-consumed (fp16 output
  staging).
- A : x1T[c,m] = sum_n x[n,c] Wq0T[n,m] + bq0[m] with x, Wq0T in fp8e4 and
  DoubleRow matmuls (2 k-tiles per instruction at 0.5 cyc/row): 64 instrs.
- B : qkT = Wqk1 @ x1 (fp32r), q/k packed 2 heads per 128-row tile in
  [d, n] layout; v natural [n, d] in [v|ones]-blocks of 65 per head.
- C : per head: scores.T = kT(64p,128) x q(64p,512) -> PSUM [128, 1024]
  double-buffered; exp on ACT -> E fp32r (the ~66us pacer);
  U = [v|ones].T @ E (N=512, fp32r full rate) -> [65, 1024] with Z in row
  64; PE-transpose per qtile ([65,128] -> [128,65]) puts U AND Z on the
  query partitions, so normalize is one reciprocal + one per-qtile
  multiply on DVE - no partition broadcast, no DMA hop; O lands natural
  [q, d] in fp32r.  T.T = (Wp0 @ O).T chains interleaved per head-pair
  under the exp stream (contraction over q -> per-pair pipelining, which
  the baseline's proj order could not do).
- D : out = T @ Wp1c.T (fp32r), fp16 staging (DMA-only consumer), DMA out.

PSUM in C: scores 2x2 banks + U 2 + transpose 1 + T.T 2 = 8 bank budget.
"""

import sys

if "/opt/trn_rl_repo" not in sys.path:
    sys.path.insert(0, "/opt/trn_rl_repo")

import numpy as np

B, N, C, H = 4, 1024, 1024, 16
HD = C // H          # 64
SCALE = HD ** -0.5
P = 128
NT = N // P          # 8 partition tiles of the 1024 axes
HPC = 8              # heads per core
VW = HD + 1          # [v | ones] block width per head

_CACHE = {}

LAST_RESULT = None   # test.py reads exec_time_ns / profile off this


def _tt_matmuls(nc, ptt, o_sb, wp0t_sb, tt_sb, pi):
    """T.T[d-block pi] = sum_q O[q, d-block] x Wp0.T[q, m] (lhsT=O natural):
    16 fp32r matmuls + 2 copy-outs, returned as thunks so the emitter can
    interleave them a few per exp step under the ACT stream."""
    import concourse.mybir as mybir

    fp32 = mybir.dt.float32

    thunks = []
    box = {}

    def mk(mch, k):
        def f():
            if k == 0:
                box[mch] = ptt.tile([P, 512], fp32, tag="tt", name="ps_tt")
            nc.tensor.matmul(
                box[mch][:],
                o_sb[:, k, 128 * pi:128 * pi + 128],
                wp0t_sb[:, k, 512 * mch:512 * mch + 512],
                start=(k == 0), stop=(k == 7),
            )
        return f

    def mkcopy(mch):
        def f():
            nc.vector.tensor_copy(
                tt_sb[:, pi, 512 * mch:512 * mch + 512], box[mch][:])
        return f

    for mch in range(2):
        for k in range(8):
            thunks.append(mk(mch, k))
        thunks.append(mkcopy(mch))
    return thunks


def _build(reps=1):
    import concourse.mybir as mybir
    import concourse.tile as tile
    from concourse import bacc

    fp32 = mybir.dt.float32
    fp32r = mybir.dt.float32r
    fp8 = mybir.dt.float8e4
    f16 = mybir.dt.float16
    Exp = mybir.ActivationFunctionType.Exp
    Ident = mybir.ActivationFunctionType.Identity
    DR = mybir.MatmulPerfMode.DoubleRow
    Add = mybir.AluOpType.add
    Mult = mybir.AluOpType.mult

    nc = bacc.Bacc("TRN2", target_bir_lowering=False, debug=False)

    def din(name, shape, dt):
        return nc.dram_tensor(name, shape, dt, kind="ExternalInput").ap()

    x_d = din("x8", [NT, P, C], fp8)
    wq0_d = din("wq0t8", [NT, P, N], fp8)
    wqk1_d = din("wqk1t_r", [NT, P, 1024], fp32r)
    wv1_d = din("wv1t_r", [NT, P, 512], fp32r)
    bq0_d = din("bq0_rep", [P, N], fp32)
    bqk1_d = din("bqk1_t", [P, 8], fp32)
    bv1_d = din("bv1_rep", [P, 512], fp32)
    wp0_d = din("wp0t_r", [NT, P, N], fp32r)
    wp1_d = din("wp1t_r", [4, P, C], fp32r)
    id65_d = din("id65_f", [P, 65], fp32)
    ones_d = din("ones_r", [P, 8], fp32r)
    out_d = nc.dram_tensor("out16", [NT, P, C], f16, kind="ExternalOutput").ap()

    with tile.TileContext(nc) as tc:
      for _rep in range(reps):
        # ---------------- pools (LIFO close order) ---------------------------
        stp_cm = tc.tile_pool(name="stage", bufs=4)
        stp = stp_cm.__enter__()
        rp_cm = tc.tile_pool(name="rp", bufs=2)
        rp = rp_cm.__enter__()

        qkv_cm = tc.tile_pool(name="qkv", bufs=1)
        qkvp = qkv_cm.__enter__()
        # q/k: [128 part = 2 heads x 64 d, head-pair, 1024 n]
        q_sb = qkvp.tile([P, 4, N], fp32r, name="q_sb")
        kt_sb = qkvp.tile([P, 4, N], fp32r, name="kt_sb")
        vpad_sb = qkvp.tile([P, NT, HPC * VW], fp32r, name="vpad_sb")
        id65_sb = qkvp.tile([P, 65], fp32, name="id65_sb")
        nc.sync.dma_start(out=id65_sb[:], in_=id65_d)
        ones_sb = qkvp.tile([P, HPC], fp32r, name="ones_sb")
        nc.sync.dma_start(out=ones_sb[:], in_=ones_d)
        for t in range(NT):
            od = vpad_sb[:, t, :].rearrange(
                "p (h j) -> p h j", h=HPC)[:, :, HD:VW]
            nc.vector.tensor_copy(od, ones_sb[:, :, None])

        # ---------------- phase A: x1T = Wq0 @ x[b] (fp8 DoubleRow) -----------
        ab_cm = tc.tile_pool(name="ab", bufs=1)
        abp = ab_cm.__enter__()
        wb_cm = tc.tile_pool(name="wb", bufs=1)
        wbp = wb_cm.__enter__()
        wa_cm = tc.tile_pool(name="wa", bufs=1)
        wap = wa_cm.__enter__()
        psa_cm = tc.tile_pool(name="psa", bufs=4, space="PSUM")
        psa = psa_cm.__enter__()

        x_sb = wap.tile([P, NT, C], fp8, name="x_sb")
        wq0t_sb = wap.tile([P, NT, N], fp8, name="wq0t_sb")
        bq0_sb = wap.tile([P, N], fp32, name="bq0_sb")
        x1t_sb = abp.tile([P, NT, N], fp32r, name="x1t_sb")
        wqk1t_sb = wbp.tile([P, NT, 1024], fp32r, name="wqk1t_sb")
        wv1t_sb = wbp.tile([P, NT, 512], fp32r, name="wv1t_sb")
        bqk1_sb = wbp.tile([P, 8], fp32, name="bqk1_sb")
        bv1_sb = wbp.tile([P, 512], fp32, name="bv1_sb")

        # interleave x/wq0t pair-DMAs so the chains can start early
        nc.sync.dma_start(out=x_sb[:, 0:2], in_=x_d[0:2])
        nc.sync.dma_start(out=wq0t_sb[:, 0:2], in_=wq0_d[0:2])
        nc.sync.dma_start(out=bq0_sb[:], in_=bq0_d)
        for t in range(1, 4):
            nc.sync.dma_start(out=x_sb[:, 2 * t:2 * t + 2], in_=x_d[2 * t:2 * t + 2])
            nc.sync.dma_start(
                out=wq0t_sb[:, 2 * t:2 * t + 2], in_=wq0_d[2 * t:2 * t + 2])
        for t in range(NT):
            nc.sync.dma_start(out=wqk1t_sb[:, t], in_=wqk1_d[t])
        nc.sync.dma_start(out=bqk1_sb[:], in_=bqk1_d)
        for t in range(NT):
            nc.sync.dma_start(out=wv1t_sb[:, t], in_=wv1_d[t])
        nc.sync.dma_start(out=bv1_sb[:], in_=bv1_d)

        # A chains: groups of 4 (2 ct x 2 mch), j-outer DoubleRow over k-pairs
        for cg in range(4):
            cts = (2 * cg, 2 * cg + 1)
            ps = {}
            for ct in cts:
                for mch in range(2):
                    ps[(ct, mch)] = psa.tile([P, 512], fp32, tag="psa", name="ps_a")
            for j in range(4):
                for ct in cts:
                    for mch in range(2):
                        nc.tensor.matmul(
                            ps[(ct, mch)][:],
                            x_sb[:, 2 * j:2 * j + 2, 128 * ct:128 * ct + 128],
                            wq0t_sb[:, 2 * j:2 * j + 2, 512 * mch:512 * mch + 512],
                            start=(j == 0), stop=(j == 3),
                            perf_mode=DR,
                        )
            for ct in cts:
                for mch in range(2):
                    msl = slice(512 * mch, 512 * mch + 512)
                    nc.vector.tensor_tensor(
                        x1t_sb[:, ct, msl], ps[(ct, mch)][:], bq0_sb[:, msl], Add)
        psa_cm.__exit__(None, None, None)
        wa_cm.__exit__(None, None, None)

        # -------- phase B: q,k (score layout) and v (natural, fp32r) ----------
        psb_cm = tc.tile_pool(name="psb", bufs=4, space="PSUM")
        psb = psb_cm.__enter__()

        for gi, dts in enumerate(((0, 1), (4, 5), (2, 3), (6, 7))):
            ps = {}
            for dt in dts:
                for mch in range(2):
                    ps[(dt, mch)] = psb.tile([P, 512], fp32, tag="psb", name="ps_b")
            for j in range(NT):
                for dt in dts:
                    for mch in range(2):
                        nc.tensor.matmul(
                            ps[(dt, mch)][:],
                            wqk1t_sb[:, j, 128 * dt:128 * dt + 128],
                            x1t_sb[:, j, 512 * mch:512 * mch + 512],
                            start=(j == 0), stop=(j == NT - 1),
                        )
            for dt in dts:
                for mch in range(2):
                    msl = slice(512 * mch, 512 * mch + 512)
                    dst = q_sb if dt < 4 else kt_sb
                    if gi < 2:
                        nc.vector.tensor_scalar_add(
                            dst[:, dt % 4, msl], ps[(dt, mch)][:],
                            bqk1_sb[:, dt:dt + 1])
                    else:
                        # later head-pair tiles: ACT is still idle here
                        nc.scalar.activation(
                            dst[:, dt % 4, msl], ps[(dt, mch)][:], Ident,
                            bias=bqk1_sb[:, dt:dt + 1])

        for ng in range(2):
            nts = range(4 * ng, 4 * ng + 4)
            ps = {}
            for nt in nts:
                ps[nt] = psb.tile([P, 512], fp32, tag="psb", name="ps_v")
            for j in range(NT):
                for nt in nts:
                    nc.tensor.matmul(
                        ps[nt][:],
                        x1t_sb[:, j, 128 * nt:128 * nt + 128],
                        wv1t_sb[:, j, :],
                        start=(j == 0), stop=(j == NT - 1),
                    )
            for nt in nts:
                vdst = vpad_sb[:, nt, :].rearrange(
                    "p (h j) -> p h j", h=HPC)[:, :, 0:HD]
                vsrc = ps[nt][:].rearrange("p (h j) -> p h j", h=HPC)
                bsrc = bv1_sb[:].rearrange("p (h j) -> p h j", h=HPC)
                nc.vector.tensor_tensor(vdst, vsrc, bsrc, Add)
        psb_cm.__exit__(None, None, None)
        wb_cm.__exit__(None, None, None)
        ab_cm.__exit__(None, None, None)

        # ------------- phase C: attention, pipelined per head -----------------
        otp_cm = tc.tile_pool(name="otp", bufs=1)
        otp = otp_cm.__enter__()
        o_sb = otp.tile([P, NT, 512], fp32r, name="o_sb")
        tt_sb = otp.tile([P, 4, N], fp32r, name="tt_sb")
        ut_sb = otp.tile([P, N], fp32, name="ut_sb")

        wd_cm = tc.tile_pool(name="wd", bufs=1)
        wdp = wd_cm.__enter__()
        wp0t_sb = wdp.tile([P, NT, N], fp32r, name="wp0t_sb")
        for t in range(NT):
            nc.sync.dma_start(out=wp0t_sb[:, t], in_=wp0_d[t])

        pst_cm = tc.tile_pool(name="pst", bufs=2, space="PSUM")
        psu_cm = tc.tile_pool(name="psu", bufs=1, space="PSUM")
        ptr_cm = tc.tile_pool(name="ptr", bufs=1, space="PSUM")
        ptt_cm = tc.tile_pool(name="ptt", bufs=1, space="PSUM")
        ep_cm = tc.tile_pool(name="ep", bufs=16)
        pst = pst_cm.__enter__()
        psu = psu_cm.__enter__()
        ptr = ptr_cm.__enter__()
        ptt = ptt_cm.__enter__()
        ep = ep_cm.__enter__()

        def u_steps(state, j):
            # U chain steps for key-tiles (2j, 2j+1) of the previous head:
            # ups[65, 1024] += [v|ones].T @ E   (rows 0..63 = U.T, row 64 = Z)
            h, et, ups = state
            vsl = slice(VW * h, VW * h + VW)
            for k in (2 * j, 2 * j + 1):
                for nch in range(2):
                    nc.tensor.matmul(
                        ups[0:VW, 512 * nch:512 * nch + 512],
                        vpad_sb[:, k, vsl],
                        et[k][:, 512 * nch:512 * nch + 512],
                        start=(k == 0), stop=(k == NT - 1),
                    )

        def u_finish(state):
            # previous head's U: PSUM [65, 1024] -> SBUF, then per qtile a
            # PE transpose puts [q,(U.T|Z)] on query partitions; normalize
            # with a reciprocal + per-qtile multiply, landing natural-layout
            # fp32r O.
            h, et, ups = state
            nc.vector.tensor_copy(ut_sb[0:VW, :], ups[0:VW, :])
            rz = rp.tile([P, 8], fp32, tag="rz", name="rz_sb")
            for qt in range(NT):
                tr = ptr.tile([P, VW], fp32, tag="tr", name="ps_tr")
                nc.tensor.matmul(
                    tr[:], ut_sb[0:VW, 128 * qt:128 * qt + 128], id65_sb[0:VW, :],
                    start=True, stop=True, is_transpose=True,
                )
                nc.vector.reciprocal(rz[:, qt:qt + 1], tr[:, HD:VW])
                nc.vector.tensor_scalar_mul(
                    o_sb[:, qt, 64 * h:64 * h + 64], tr[:, 0:HD],
                    rz[:, qt:qt + 1])

        prev = None
        ttq = []  # pending proj-T.T thunks, drained a few per exp step
        for h in range(HPC):
            tp, a = h // 2, h % 2
            psl = slice(64 * a, 64 * a + 64)
            et = [None] * NT
            for j in range(4):
                if prev is not None:
                    u_steps(prev, j)
                for i in range(2):
                    mt = 2 * j + i
                    # [128,1024] score tiles, bufs=2: the next tile's scores
                    # run while ACT exps the previous one (ACT stays saturated)
                    ps = pst.tile([P, 1024], fp32, tag="st", name="ps_st")
                    for mch in range(2):
                        nc.tensor.matmul(
                            ps[:, 512 * mch:512 * mch + 512],
                            kt_sb[psl, tp, 128 * mt:128 * mt + 128],
                            q_sb[psl, tp, 512 * mch:512 * mch + 512],
                            start=True, stop=True,
                        )
                    etj = ep.tile([P, 1024], fp32r, tag="e", name="e_sb")
                    nc.scalar.activation(etj[:], ps[:], Exp, scale=SCALE)
                    et[mt] = etj
                    for _ in range(min(2, len(ttq))):
                        ttq.pop(0)()
            if prev is not None:
                u_finish(prev)
                if prev[0] % 2 == 1:
                    ttq += _tt_matmuls(nc, ptt, o_sb, wp0t_sb, tt_sb, prev[0] // 2)
            ups = psu.tile([P, 1024], fp32, tag="u", name="ps_u")
            prev = (h, et, ups)
        for j in range(4):
            u_steps(prev, j)
            for _ in range(min(4, len(ttq))):
                ttq.pop(0)()
        u_finish(prev)
        ttq += _tt_matmuls(nc, ptt, o_sb, wp0t_sb, tt_sb, 3)
        for fn in ttq:
            fn()
        ep_cm.__exit__(None, None, None)
        ptt_cm.__exit__(None, None, None)
        ptr_cm.__exit__(None, None, None)
        psu_cm.__exit__(None, None, None)
        pst_cm.__exit__(None, None, None)

        # ---------- phase D: out = T @ Wp1c.T (fp32r) -------------------------
        wd2_cm = tc.tile_pool(name="wd2", bufs=1)
        wd2p = wd2_cm.__enter__()
        wp1t_sb = wd2p.tile([P, 4, C], fp32r, name="wp1t_sb")
        for t in range(4):
            nc.sync.dma_start(out=wp1t_sb[:, t], in_=wp1_d[t])
        psd_cm = tc.tile_pool(name="psd", bufs=4, space="PSUM")
        psd = psd_cm.__enter__()
        for mt in range(NT):
            for dch in range(2):
                dsl = slice(512 * dch, 512 * dch + 512)
                ps = psd.tile([P, 512], fp32, tag="psd", name="ps_o")
                for kd in range(4):
                    nc.tensor.matmul(
                        ps[:],
                        tt_sb[:, kd, 128 * mt:128 * mt + 128],
                        wp1t_sb[:, kd, dsl],
                        start=(kd == 0), stop=(kd == 3),
                    )
                ostage = stp.tile([P, 512], f16, tag="ost", name="out_stage")
                if mt % 2 == 0:
                    nc.vector.tensor_copy(ostage[:], ps[:])
                else:
                    nc.scalar.copy(ostage[:], ps[:])
                nc.sync.dma_start(out=out_d[mt, :, dsl], in_=ostage[:])
        psd_cm.__exit__(None, None, None)
        wd2_cm.__exit__(None, None, None)
        wd_cm.__exit__(None, None, None)
        otp_cm.__exit__(None, None, None)
        qkv_cm.__exit__(None, None, None)
        rp_cm.__exit__(None, None, None)
        stp_cm.__exit__(None, None, None)

    nc.compile()
    return nc


def _get_nc(reps=1):
    key = ("nc", reps)
    if key not in _CACHE:
        _CACHE[key] = _build(reps)
    return _CACHE[key]


def _in_maps(x, Wq0, bq0, Wq1, bq1, Wp0, bp0, Wp1, bp1):
    import ml_dtypes

    f = np.float32
    e4 = ml_dtypes.float8_e4m3
    x = np.asarray(x, f)
    Wq0 = np.asarray(Wq0, f); bq0 = np.asarray(bq0, f)
    Wq1 = np.asarray(Wq1, f); bq1 = np.asarray(bq1, f)
    Wp0 = np.asarray(Wp0, f); Wp1 = np.asarray(Wp1, f)
    wq0t8 = np.ascontiguousarray(Wq0.T.reshape(NT, P, N)).astype(e4)
    wp0t = np.ascontiguousarray(Wp0.T.reshape(NT, P, N))
    bq0r = np.ascontiguousarray(np.broadcast_to(bq0, (P, N)))
    id65 = np.zeros((P, 65), f)
    id65[:65, :] = np.eye(65, dtype=f)
    maps = []
    for core in range(8):
        b, g = core // 2, core % 2
        # natural layout: qk tile dt<4 = q head-pair (2dt, 2dt+1), dt>=4 = k
        perm = np.concatenate([
            np.arange(512 * g, 512 * g + 512),
            np.arange(C + 512 * g, C + 512 * g + 512)])
        wqk1 = Wq1[perm]                                      # (1024 d', 1024 c)
        vs = slice(2 * C + 512 * g, 2 * C + 512 * g + 512)
        m = {
            "x8": np.ascontiguousarray(x[b].reshape(NT, P, C)).astype(e4),
            "wq0t8": wq0t8,
            "wqk1t_r": np.ascontiguousarray(wqk1.T.reshape(NT, P, 1024)),
            "wv1t_r": np.ascontiguousarray(Wq1[vs].T.reshape(NT, P, 512)),
            "bq0_rep": bq0r,
            "bqk1_t": np.ascontiguousarray(bq1[perm].reshape(8, P).T),
            "bv1_rep": np.ascontiguousarray(np.broadcast_to(bq1[vs], (P, 512))),
            "wp0t_r": wp0t,
            "wp1t_r": np.ascontiguousarray(
                Wp1[:, 512 * g:512 * g + 512].T.reshape(4, P, C)),
            "id65_f": id65,
            "ones_r": np.ones((P, 8), f),
        }
        maps.append(m)
    return maps


def kernel(x, Wq0, bq0, Wq1, bq1, Wp0, bp0, Wp1, bp1):
    global LAST_RESULT
    import os

    # The SPMD execute path needs jax's axon PJRT backend; a harness that
    # pinned JAX_PLATFORMS=cpu (common for running the jax reference) would
    # otherwise hide the NeuronCores from this process.
    if "axon" not in os.environ.get("JAX_PLATFORMS", "axon"):
        os.environ.pop("JAX_PLATFORMS", None)
    # This container lacks antenv.axon_hooks, so the BASS_TRACE=1 NTFF path
    # in run_bass_kernel_spmd raises ModuleNotFoundError. Force tracing off
    # (a crash would otherwise replace a working run).
    os.environ["BASS_NEVER_TRACE"] = "1"
    from concourse.bass_utils import run_bass_kernel_spmd

    nc = _get_nc()
    maps = _in_maps(x, Wq0, bq0, Wq1, bq1, Wp0, bp0, Wp1, bp1)
    res = run_bass_kernel_spmd(nc, maps, list(range(8)))
    LAST_RESULT = res
    parts = [np.asarray(r["out16"], np.float32).reshape(N, C)
             for r in res.results]
    f = np.float32
    bp0 = np.asarray(bp0, f); bp1 = np.asarray(bp1, f)
    Wp1 = np.asarray(Wp1, f)
    bias = np.outer(bp0, Wp1.sum(axis=1)) + bp1[None, :]
    out = np.stack(
        [parts[2 * b] + parts[2 * b + 1] + bias for b in range(B)], 0)
    return out.astype(f)


# revision 41
# speedup vs baseline: 1.0282x; 1.0282x over previous
"""Trainium2 Bass kernel for nn_Attention_56487409877769.

NdLinear-qkv -> 16-head attention -> NdLinear-proj, B=4 N=1024 C=1024 H=16.

Sharding: 8 cores = (batch b, head-group g) with b=core//2, g=core%2.
Each core handles batch b and its 8 heads (qkv channel slice 512g:512g+512).
The proj channel matmul is a partial sum over the core's channel slice; the
host adds the two partials per batch plus a rank-1 bias term (the NdLinear
proj biases commute: out = Wp0 @ O @ Wp1c.T + outer(bp0, Wp1.sum(1)) + bp1).

v3 design notes:
- On-device dtype-SIZE-converting writes (fp32 PSUM -> fp16/fp8 SBUF) are
  broken for downstream PE consumers in this stack (verified by minimal
  repro; DMA readers see such tiles fine).  Therefore every PE-consumed
  tile that is produced on-device is fp32r (same-size as its fp32 source);
  narrow dtypes are used only where the data is HOST-prepared and
  DMA-written (phase-A fp8 DoubleRow inputs) or DMA-consumed (fp16 output
  staging).
- A : x1T[c,m] = sum_n x[n,c] Wq0T[n,m] + bq0[m] with x, Wq0T in fp8e4 and
  DoubleRow matmuls (2 k-tiles per instruction at 0.5 cyc/row): 64 instrs.
- B : qkT = Wqk1 @ x1 (fp32r), q/k packed 2 heads per 128-row tile in
  [d, n] layout; v natural [n, d] in [v|ones]-blocks of 65 per head.
- C : per head: scores.T = kT(64p,128) x q(64p,512) -> PSUM [128, 1024]
  double-buffered; exp on ACT -> E fp32r (the ~66us pacer);
  U = [v|ones].T @ E (N=512, fp32r full rate) -> [65, 1024] with Z in row
  64; PE-transpose per qtile ([65,128] -> [128,65]) puts U AND Z on the
  query partitions, so normalize is one reciprocal + one per-qtile
  multiply on DVE - no partition broadcast, no DMA hop; O lands natural
  [q, d] in fp32r.  T.T = (Wp0 @ O).T chains interleaved per head-pair
  under the exp stream (contraction over q -> per-pair pipelining, which
  the baseline's proj order could not do).
- D : out = T @ Wp1c.T (fp32r), fp16 staging (DMA-only consumer), DMA out.

PSUM in C: scores 2x2 banks + U 2 + transpose 1 + T.T 2 = 8 bank budget.
"""

import sys

if "/opt/trn_rl_repo" not in sys.path:
    sys.path.insert(0, "/opt/trn_rl_repo")

import numpy as np

B, N, C, H = 4, 1024, 1024, 16
HD = C // H          # 64
SCALE = HD ** -0.5
P = 128
NT = N // P          # 8 partition tiles of the 1024 axes
HPC = 8              # heads per core
VW = HD + 1          # [v | ones] block width per head

_CACHE = {}

LAST_RESULT = None   # test.py reads exec_time_ns / profile off this


def _tt_matmuls(nc, ptt, o_sb, wp0t_sb, tt_sb, pi):
    """T.T[d-block pi] = sum_q O[q, d-block] x Wp0.T[q, m] (lhsT=O natural):
    16 fp32r matmuls + 2 copy-outs, returned as thunks so the emitter can
    interleave them a few per exp step under the ACT stream."""
    import concourse.mybir as mybir

    fp32 = mybir.dt.float32

    thunks = []
    box = {}

    def mk(mch, k):
        def f():
            if k == 0:
                box[mch] = ptt.tile([P, 512], fp32, tag="tt", name="ps_tt")
            nc.tensor.matmul(
                box[mch][:],
                o_sb[:, k, 128 * pi:128 * pi + 128],
                wp0t_sb[:, k, 512 * mch:512 * mch + 512],
                start=(k == 0), stop=(k == 7),
            )
        return f

    def mkcopy(mch):
        def f():
            nc.vector.tensor_copy(
                tt_sb[:, pi, 512 * mch:512 * mch + 512], box[mch][:])
        return f

    for mch in range(2):
        for k in range(8):
            thunks.append(mk(mch, k))
        thunks.append(mkcopy(mch))
    return thunks


def _build(reps=1, stop=None):
    import concourse.mybir as mybir
    import concourse.tile as tile
    from concourse import bacc

    fp32 = mybir.dt.float32
    fp32r = mybir.dt.float32r
    fp8 = mybir.dt.float8e4
    f16 = mybir.dt.float16
    Exp = mybir.ActivationFunctionType.Exp
    Ident = mybir.ActivationFunctionType.Identity
    DR = mybir.MatmulPerfMode.DoubleRow
    Add = mybir.AluOpType.add
    Mult = mybir.AluOpType.mult

    nc = bacc.Bacc("TRN2", target_bir_lowering=False, debug=False)

    def din(name, shape, dt):
        return nc.dram_tensor(name, shape, dt, kind="ExternalInput").ap()

    x_d = din("x8", [NT, P, C], fp8)
    wq0_d = din("wq0t8", [NT, P, N], fp8)
    wqk1_d = din("wqk1t_r", [NT, P, 1024], fp32r)
    wv1_d = din("wv1t_r", [NT, P, 512], fp32r)
    bq0_d = din("bq0_rep", [P, N], fp32)
    bqk1_d = din("bqk1_t", [P, 8], fp32)
    bv1_d = din("bv1_rep", [P, 512], fp32)
    wp0_d = din("wp0t_r", [NT, P, N], fp32r)
    wp1_d = din("wp1t_r", [4, P, C], fp32r)
    id65_d = din("id65_f", [P, 65], fp32)
    ones_d = din("ones_r", [P, 8], fp32r)
    out_d = nc.dram_tensor("out16", [NT, P, C], f16, kind="ExternalOutput").ap()

    with tile.TileContext(nc) as tc:
      for _rep in range(reps):
        # ---------------- pools (LIFO close order) ---------------------------
        stp_cm = tc.tile_pool(name="stage", bufs=4)
        stp = stp_cm.__enter__()
        rp_cm = tc.tile_pool(name="rp", bufs=2)
        rp = rp_cm.__enter__()

        qkv_cm = tc.tile_pool(name="qkv", bufs=1)
        qkvp = qkv_cm.__enter__()
        # q/k: [128 part = 2 heads x 64 d, head-pair, 1024 n]
        q_sb = qkvp.tile([P, 4, N], fp32r, name="q_sb")
        kt_sb = qkvp.tile([P, 4, N], fp32r, name="kt_sb")
        vpad_sb = qkvp.tile([P, NT, HPC * VW], fp32r, name="vpad_sb")
        id65_sb = qkvp.tile([P, 65], fp32, name="id65_sb")
        nc.sync.dma_start(out=id65_sb[:], in_=id65_d)
        ones_sb = qkvp.tile([P, HPC], fp32r, name="ones_sb")
        nc.sync.dma_start(out=ones_sb[:], in_=ones_d)
        for t in range(NT):
            od = vpad_sb[:, t, :].rearrange(
                "p (h j) -> p h j", h=HPC)[:, :, HD:VW]
            nc.vector.tensor_copy(od, ones_sb[:, :, None])

        # ---------------- phase A: x1T = Wq0 @ x[b] (fp8 DoubleRow) -----------
        ab_cm = tc.tile_pool(name="ab", bufs=1)
        abp = ab_cm.__enter__()
        wb_cm = tc.tile_pool(name="wb", bufs=1)
        wbp = wb_cm.__enter__()
        wa_cm = tc.tile_pool(name="wa", bufs=1)
        wap = wa_cm.__enter__()
        psb_cm = tc.tile_pool(name="psb", bufs=4, space="PSUM")
        psb = psb_cm.__enter__()
        psa_cm = tc.tile_pool(name="psa", bufs=4, space="PSUM")
        psa = psa_cm.__enter__()

        x_sb = wap.tile([P, NT, C], fp8, name="x_sb")
        wq0t_sb = wap.tile([P, NT, N], fp8, name="wq0t_sb")
        bq0_sb = wap.tile([P, N], fp32, name="bq0_sb")
        x1t_sb = abp.tile([P, NT, N], fp32r, name="x1t_sb")
        wqk1t_sb = wbp.tile([P, NT, 1024], fp32r, name="wqk1t_sb")
        wv1t_sb = wbp.tile([P, NT, 512], fp32r, name="wv1t_sb")
        bqk1_sb = wbp.tile([P, 8], fp32, name="bqk1_sb")
        bv1_sb = wbp.tile([P, 512], fp32, name="bv1_sb")

        # interleave x/wq0t pair-DMAs so the chains can start early
        nc.sync.dma_start(out=x_sb[:, 0:2], in_=x_d[0:2])
        nc.sync.dma_start(out=wq0t_sb[:, 0:2], in_=wq0_d[0:2])
        nc.sync.dma_start(out=bq0_sb[:], in_=bq0_d)
        for t in range(1, 4):
            nc.sync.dma_start(out=x_sb[:, 2 * t:2 * t + 2], in_=x_d[2 * t:2 * t + 2])
            nc.sync.dma_start(
                out=wq0t_sb[:, 2 * t:2 * t + 2], in_=wq0_d[2 * t:2 * t + 2])
        for t in range(NT):
            nc.sync.dma_start(out=wqk1t_sb[:, t], in_=wqk1_d[t])
        nc.sync.dma_start(out=bqk1_sb[:], in_=bqk1_d)
        for t in range(NT):
            nc.sync.dma_start(out=wv1t_sb[:, t], in_=wv1_d[t])
        nc.sync.dma_start(out=bv1_sb[:], in_=bv1_d)

        # B-chain helper: emit one j-step (4 matmuls) of a 4-chain group
        def b_group(dts_or_nts, kind):
            ps = {}

            def step(j):
                if j == 0:
                    for key in (
                        [(dt, mch) for dt in dts_or_nts for mch in range(2)]
                        if kind == "qk" else list(dts_or_nts)):
                        ps[key] = psb.tile(
                            [P, 512], fp32, tag="psb", name="ps_b")
                if kind == "qk":
                    for dt in dts_or_nts:
                        for mch in range(2):
                            nc.tensor.matmul(
                                ps[(dt, mch)][:],
                                wqk1t_sb[:, j, 128 * dt:128 * dt + 128],
                                x1t_sb[:, j, 512 * mch:512 * mch + 512],
                                start=(j == 0), stop=(j == NT - 1),
                            )
                else:
                    for nt in dts_or_nts:
                        nc.tensor.matmul(
                            ps[nt][:],
                            x1t_sb[:, j, 128 * nt:128 * nt + 128],
                            wv1t_sb[:, j, :],
                            start=(j == 0), stop=(j == NT - 1),
                        )

            def copies(engine_dve):
                if kind == "qk":
                    for dt in dts_or_nts:
                        for mch in range(2):
                            msl = slice(512 * mch, 512 * mch + 512)
                            dst = q_sb if dt < 4 else kt_sb
                            if engine_dve:
                                nc.vector.tensor_scalar_add(
                                    dst[:, dt % 4, msl], ps[(dt, mch)][:],
                                    bqk1_sb[:, dt:dt + 1])
                            else:
                                nc.scalar.activation(
                                    dst[:, dt % 4, msl], ps[(dt, mch)][:],
                                    Ident, bias=bqk1_sb[:, dt:dt + 1])
                else:
                    for nt in dts_or_nts:
                        vdst = vpad_sb[:, nt, :].rearrange(
                            "p (h j) -> p h j", h=HPC)[:, :, 0:HD]
                        vsrc = ps[nt][:].rearrange("p (h j) -> p h j", h=HPC)
                        bsrc = bv1_sb[:].rearrange("p (h j) -> p h j", h=HPC)
                        nc.vector.tensor_tensor(vdst, vsrc, bsrc, Add)

            return step, copies

        q0_step, q0_copies = b_group((0, 1), "qk")

        # A chains: groups of 4 (2 ct x 2 mch), j-outer DoubleRow over k-pairs;
        # B's q0 group (which only needs x1t tiles progressively) is
        # interleaved so the PE never idles waiting for the wqk1t DMAs.
        for cg in range(4):
            cts = (2 * cg, 2 * cg + 1)
            ps = {}
            for ct in cts:
                for mch in range(2):
                    ps[(ct, mch)] = psa.tile([P, 512], fp32, tag="psa", name="ps_a")
            for j in range(4):
                for ct in cts:
                    for mch in range(2):
                        nc.tensor.matmul(
                            ps[(ct, mch)][:],
                            x_sb[:, 2 * j:2 * j + 2, 128 * ct:128 * ct + 128],
                            wq0t_sb[:, 2 * j:2 * j + 2, 512 * mch:512 * mch + 512],
                            start=(j == 0), stop=(j == 3),
                            perf_mode=DR,
                        )
            for ct in cts:
                for mch in range(2):
                    msl = slice(512 * mch, 512 * mch + 512)
                    nc.vector.tensor_tensor(
                        x1t_sb[:, ct, msl], ps[(ct, mch)][:], bq0_sb[:, msl], Add)
            if cg >= 1:
                q0_step(2 * (cg - 1))
                q0_step(2 * (cg - 1) + 1)
        q0_step(6)
        q0_step(7)
        q0_copies(True)
        psa_cm.__exit__(None, None, None)
        wa_cm.__exit__(None, None, None)

        # -------- phase B: remaining groups, serial ---------------------------
        for dts_or_nts, kind, dve in (
                ((4, 5), "qk", True), ((2, 3), "qk", True),
                ((6, 7), "qk", False),
                ((0, 1, 2, 3), "v", True), ((4, 5, 6, 7), "v", True)):
            stepf, copyf = b_group(dts_or_nts, kind)
            for j in range(NT):
                stepf(j)
            copyf(dve)
        psb_cm.__exit__(None, None, None)
        wb_cm.__exit__(None, None, None)
        ab_cm.__exit__(None, None, None)
        if stop == "b":
            qkv_cm.__exit__(None, None, None)
            rp_cm.__exit__(None, None, None)
            stp_cm.__exit__(None, None, None)
            continue

        # ------------- phase C: attention, pipelined per head -----------------
        otp_cm = tc.tile_pool(name="otp", bufs=1)
        otp = otp_cm.__enter__()
        o_sb = otp.tile([P, NT, 512], fp32r, name="o_sb")
        tt_sb = otp.tile([P, 4, N], fp32r, name="tt_sb")
        ut_sb = otp.tile([P, N], fp32, name="ut_sb")

        wd_cm = tc.tile_pool(name="wd", bufs=1)
        wdp = wd_cm.__enter__()
        wp0t_sb = wdp.tile([P, NT, N], fp32r, name="wp0t_sb")
        for t in range(NT):
            nc.sync.dma_start(out=wp0t_sb[:, t], in_=wp0_d[t])

        pst_cm = tc.tile_pool(name="pst", bufs=2, space="PSUM")
        psu_cm = tc.tile_pool(name="psu", bufs=1, space="PSUM")
        ptr_cm = tc.tile_pool(name="ptr", bufs=1, space="PSUM")
        ptt_cm = tc.tile_pool(name="ptt", bufs=1, space="PSUM")
        ep_cm = tc.tile_pool(name="ep", bufs=16)
        pst = pst_cm.__enter__()
        psu = psu_cm.__enter__()
        ptr = ptr_cm.__enter__()
        ptt = ptt_cm.__enter__()
        ep = ep_cm.__enter__()

        def u_steps(state, j):
            # U chain steps for key-tiles (2j, 2j+1) of the previous head:
            # ups[65, 1024] += [v|ones].T @ E   (rows 0..63 = U.T, row 64 = Z)
            h, et, ups = state
            vsl = slice(VW * h, VW * h + VW)
            for k in (2 * j, 2 * j + 1):
                for nch in range(2):
                    nc.tensor.matmul(
                        ups[0:VW, 512 * nch:512 * nch + 512],
                        vpad_sb[:, k, vsl],
                        et[k][:, 512 * nch:512 * nch + 512],
                        start=(k == 0), stop=(k == NT - 1),
                    )

        def u_finish(state):
            # previous head's U: PSUM [65, 1024] -> SBUF, then per qtile a
            # PE transpose puts [q,(U.T|Z)] on query partitions; normalize
            # with a reciprocal + per-qtile multiply, landing natural-layout
            # fp32r O.
            h, et, ups = state
            nc.vector.tensor_copy(ut_sb[0:VW, :], ups[0:VW, :])
            rz = rp.tile([P, 8], fp32, tag="rz", name="rz_sb")
            for qt in range(NT):
                tr = ptr.tile([P, VW], fp32, tag="tr", name="ps_tr")
                nc.tensor.matmul(
                    tr[:], ut_sb[0:VW, 128 * qt:128 * qt + 128], id65_sb[0:VW, :],
                    start=True, stop=True, is_transpose=True,
                )
                nc.vector.reciprocal(rz[:, qt:qt + 1], tr[:, HD:VW])
                nc.vector.tensor_scalar_mul(
                    o_sb[:, qt, 64 * h:64 * h + 64], tr[:, 0:HD],
                    rz[:, qt:qt + 1])

        prev = None
        ttq = []  # pending proj-T.T thunks, drained a few per exp step
        for h in range(HPC):
            tp, a = h // 2, h % 2
            psl = slice(64 * a, 64 * a + 64)
            et = [None] * NT
            for j in range(4):
                if prev is not None:
                    u_steps(prev, j)
                for i in range(2):
                    mt = 2 * j + i
                    # [128,1024] score tiles, bufs=2: the next tile's scores
                    # run while ACT exps the previous one (ACT stays saturated)
                    ps = pst.tile([P, 1024], fp32, tag="st", name="ps_st")
                    for mch in range(2):
                        nc.tensor.matmul(
                            ps[:, 512 * mch:512 * mch + 512],
                            kt_sb[psl, tp, 128 * mt:128 * mt + 128],
                            q_sb[psl, tp, 512 * mch:512 * mch + 512],
                            start=True, stop=True,
                        )
                    etj = ep.tile([P, 1024], fp32r, tag="e", name="e_sb")
                    nc.scalar.activation(etj[:], ps[:], Exp, scale=SCALE)
                    et[mt] = etj
                    for _ in range(min(2, len(ttq))):
                        ttq.pop(0)()
            if prev is not None:
                u_finish(prev)
                if prev[0] % 2 == 1:
                    ttq += _tt_matmuls(nc, ptt, o_sb, wp0t_sb, tt_sb, prev[0] // 2)
            ups = psu.tile([P, 1024], fp32, tag="u", name="ps_u")
            prev = (h, et, ups)
        for j in range(4):
            u_steps(prev, j)
            for _ in range(min(4, len(ttq))):
                ttq.pop(0)()
        u_finish(prev)
        ttq += _tt_matmuls(nc, ptt, o_sb, wp0t_sb, tt_sb, 3)
        for fn in ttq:
            fn()
        ep_cm.__exit__(None, None, None)
        ptt_cm.__exit__(None, None, None)
        ptr_cm.__exit__(None, None, None)
        psu_cm.__exit__(None, None, None)
        pst_cm.__exit__(None, None, None)
        if stop == "c":
            wd_cm.__exit__(None, None, None)
            otp_cm.__exit__(None, None, None)
            qkv_cm.__exit__(None, None, None)
            rp_cm.__exit__(None, None, None)
            stp_cm.__exit__(None, None, None)
            continue

        # ---------- phase D: out = T @ Wp1c.T (fp32r) -------------------------
        wd2_cm = tc.tile_pool(name="wd2", bufs=1)
        wd2p = wd2_cm.__enter__()
        wp1t_sb = wd2p.tile([P, 4, C], fp32r, name="wp1t_sb")
        for t in range(4):
            nc.sync.dma_start(out=wp1t_sb[:, t], in_=wp1_d[t])
        psd_cm = tc.tile_pool(name="psd", bufs=4, space="PSUM")
        psd = psd_cm.__enter__()
        for mt in range(NT):
            for dch in range(2):
                dsl = slice(512 * dch, 512 * dch + 512)
                ps = psd.tile([P, 512], fp32, tag="psd", name="ps_o")
                for kd in range(4):
                    nc.tensor.matmul(
                        ps[:],
                        tt_sb[:, kd, 128 * mt:128 * mt + 128],
                        wp1t_sb[:, kd, dsl],
                        start=(kd == 0), stop=(kd == 3),
                    )
                ostage = stp.tile([P, 512], f16, tag="ost", name="out_stage")
                if mt % 2 == 0:
                    nc.vector.tensor_copy(ostage[:], ps[:])
                else:
                    nc.scalar.copy(ostage[:], ps[:])
                nc.sync.dma_start(out=out_d[mt, :, dsl], in_=ostage[:])
        psd_cm.__exit__(None, None, None)
        wd2_cm.__exit__(None, None, None)
        wd_cm.__exit__(None, None, None)
        otp_cm.__exit__(None, None, None)
        qkv_cm.__exit__(None, None, None)
        rp_cm.__exit__(None, None, None)
        stp_cm.__exit__(None, None, None)

    nc.compile()
    return nc


def _get_nc(reps=1):
    key = ("nc", reps)
    if key not in _CACHE:
        _CACHE[key] = _build(reps)
    return _CACHE[key]


def _in_maps(x, Wq0, bq0, Wq1, bq1, Wp0, bp0, Wp1, bp1):
    import ml_dtypes

    f = np.float32
    e4 = ml_dtypes.float8_e4m3
    x = np.asarray(x, f)
    Wq0 = np.asarray(Wq0, f); bq0 = np.asarray(bq0, f)
    Wq1 = np.asarray(Wq1, f); bq1 = np.asarray(bq1, f)
    Wp0 = np.asarray(Wp0, f); Wp1 = np.asarray(Wp1, f)
    wq0t8 = np.ascontiguousarray(Wq0.T.reshape(NT, P, N)).astype(e4)
    wp0t = np.ascontiguousarray(Wp0.T.reshape(NT, P, N))
    bq0r = np.ascontiguousarray(np.broadcast_to(bq0, (P, N)))
    id65 = np.zeros((P, 65), f)
    id65[:65, :] = np.eye(65, dtype=f)
    maps = []
    for core in range(8):
        b, g = core // 2, core % 2
        # natural layout: qk tile dt<4 = q head-pair (2dt, 2dt+1), dt>=4 = k
        perm = np.concatenate([
            np.arange(512 * g, 512 * g + 512),
            np.arange(C + 512 * g, C + 512 * g + 512)])
        wqk1 = Wq1[perm]                                      # (1024 d', 1024 c)
        vs = slice(2 * C + 512 * g, 2 * C + 512 * g + 512)
        m = {
            "x8": np.ascontiguousarray(x[b].reshape(NT, P, C)).astype(e4),
            "wq0t8": wq0t8,
            "wqk1t_r": np.ascontiguousarray(wqk1.T.reshape(NT, P, 1024)),
            "wv1t_r": np.ascontiguousarray(Wq1[vs].T.reshape(NT, P, 512)),
            "bq0_rep": bq0r,
            "bqk1_t": np.ascontiguousarray(bq1[perm].reshape(8, P).T),
            "bv1_rep": np.ascontiguousarray(np.broadcast_to(bq1[vs], (P, 512))),
            "wp0t_r": wp0t,
            "wp1t_r": np.ascontiguousarray(
                Wp1[:, 512 * g:512 * g + 512].T.reshape(4, P, C)),
            "id65_f": id65,
            "ones_r": np.ones((P, 8), f),
        }
        maps.append(m)
    return maps


def kernel(x, Wq0, bq0, Wq1, bq1, Wp0, bp0, Wp1, bp1):
    global LAST_RESULT
    import os

    # The SPMD execute path needs jax's axon PJRT backend; a harness that
    # pinned JAX_PLATFORMS=cpu (common for running the jax reference) would
    # otherwise hide the NeuronCores from this process.
    if "axon" not in os.environ.get("JAX_PLATFORMS", "axon"):
        os.environ.pop("JAX_PLATFORMS", None)
    # This container lacks antenv.axon_hooks, so the BASS_TRACE=1 NTFF path
    # in run_bass_kernel_spmd raises ModuleNotFoundError. Force tracing off
    # (a crash would otherwise replace a working run).
    os.environ["BASS_NEVER_TRACE"] = "1"
    from concourse.bass_utils import run_bass_kernel_spmd

    nc = _get_nc()
    maps = _in_maps(x, Wq0, bq0, Wq1, bq1, Wp0, bp0, Wp1, bp1)
    res = run_bass_kernel_spmd(nc, maps, list(range(8)))
    LAST_RESULT = res
    parts = [np.asarray(r["out16"], np.float32).reshape(N, C)
             for r in res.results]
    f = np.float32
    bp0 = np.asarray(bp0, f); bp1 = np.asarray(bp1, f)
    Wp1 = np.asarray(Wp1, f)
    bias = np.outer(bp0, Wp1.sum(axis=1)) + bp1[None, :]
    out = np.stack(
        [parts[2 * b] + parts[2 * b + 1] + bias for b in range(B)], 0)
    return out.astype(f)
